# revision 4
# baseline (speedup 1.0000x reference)
"""Trainium2 Bass kernel for nn_DetectSpikes (spatiotemporal NMS spike detection).

kernel(traces [150000,384] f32, channel_locations [384,2] f32) ->
(times int64 [100000], chans int32 [100000]) matching the reference exactly.

Detection rule (x_inv = -traces): (n, m) is a detection iff x_inv >= 3.0,
time margin, and x_inv >= max over adj(m) x [n-15, n+15] (ties pass).

Device (8 cores, time-sharded with halo, SPMD). All device math runs in the
min-domain (raw traces, bf16; block MIN equals x_inv block max, negated):
  - Host pre-transposes traces to [chan, time] bf16: no PE transpose, half
    the HBM traffic of f32.
  - 8-sample block minima B8 per channel via a 3-step min tree (DVE).
  - P2 = pairwise min of B8; c8 = 5-block cover min (bf16 cover table for
    the host sure-check).
  - LSE screen at block resolution: wp = exp(-C*(P2+B0)) (ACT),
    ws = two-offset 16-sample grid sum, zs = adjacency matmul (PE, bf16),
    zsE = zs + floor (ACT copy from PSUM, bf16 out).
  Outputs per block: B8, c8, zsE (all bf16) -- 3/8 the input volume.
Host: flags the ~1% of blocks with B8 >= ln(zsE)/C + B0 - guard, screens
  their 8 samples against the same bound (guards cover every bf16/exp
  error), classifies sure vs ambiguous via the c8 cover table, and
  resolves the ambiguous ones exactly from raw f32 traces. Output exact.
"""

import time

import numpy as np
import ml_dtypes

import concourse.bass as bass
import concourse.tile as tile
from concourse import bacc, mybir
from concourse.bass_utils import run_bass_kernel_spmd

# ---- problem constants ----
N, M = 150000, 384
TR = 15
THR = 3.0
MARGIN = 20
RADIUS = 100.0
MAX_DET = 100000
NCORES = 8
INT = N // NCORES             # 18750

T_LOC = 18944                 # 8*2368 >= INT + 2*TR, multiple of 32
NB8 = T_LOC // 8              # 2368
SCS = [296, 592, 592, 444, 296, 148]  # super-chunk sizes (blocks); sum = NB8
SCO = [0, 296, 888, 1480, 1924, 2220]  # block offsets
NSC = len(SCS)
MMW = 148                     # matmul chunk (one PSUM bank)

PAD = 240.0                   # min-domain pad (= -240 in x_inv domain)
C_LN = float(32.0 * np.log(2.0))
B0 = 5.4
THR_FLOOR = 2.93
E_FLOOR = float(np.exp(C_LN * (THR_FLOOR - B0)))
SLACK_SURE = 0.02             # host sure-check slack for bf16 rounding
D_EPS = 0.004                 # exp/matmul/bf16-zse error stack (host guard)

POOL_WS = False               # ws add runs on DVE (shorter chain)

_F32 = mybir.dt.float32
_BF16 = mybir.dt.bfloat16
_U8 = mybir.dt.uint8


def build_program():
    nc = bacc.Bacc(
        "TRN2", target_bir_lowering=False, debug=False, enable_asserts=False,
        num_devices=NCORES,
    )
    xs = nc.dram_tensor("xs", [3, 128, T_LOC], _BF16, kind="ExternalInput")
    wadj = nc.dram_tensor("wadj", [3, 3, 128, 128], _BF16, kind="ExternalInput")
    zs_d = nc.dram_tensor("zse", [3, 128, NB8], _BF16, kind="ExternalOutput")
    b8_d = nc.dram_tensor("b8", [3, 128, NB8], _BF16, kind="ExternalOutput")

    BEXP = float(np.float32(-C_LN * B0))

    from contextlib import ExitStack
    with tile.TileContext(nc) as tc, ExitStack() as ctx:
        consts = ctx.enter_context(tc.tile_pool(name="consts", bufs=1))
        persist = ctx.enter_context(tc.tile_pool(name="persist", bufs=1))
        xp = ctx.enter_context(tc.tile_pool(name="xp", bufs=5))
        trp = ctx.enter_context(tc.tile_pool(name="trp", bufs=2))
        tabp = ctx.enter_context(tc.tile_pool(name="tabp", bufs=2))
        wsp = ctx.enter_context(tc.tile_pool(name="wsp", bufs=6))
        zsep = ctx.enter_context(tc.tile_pool(name="zsep", bufs=2))
        outp = ctx.enter_context(tc.tile_pool(name="outp", bufs=3))
        psp = ctx.enter_context(tc.tile_pool(name="psp", bufs=2, space="PSUM"))

        # adjacency blocks [c_sb partitions, c_db], one standalone tile each
        wt = {}
        for sb in range(3):
            for db in range(3):
                wtile = consts.tile([128, 128], _BF16, tag=f"wa{sb}{db}")
                wt[(sb, db)] = wtile

        def load_wadj():
            for sb in range(3):
                for db in range(3):
                    nc.sync.dma_start(wt[(sb, db)][:], wadj.ap()[sb, db, :, :])
        bexp = consts.tile([128, 1], _F32, tag="bexp")
        nc.vector.memset(bexp[:], BEXP)

        # B8 block minima, padded by 2 cols each side (col j <-> block j-2)
        b8 = {}
        for db in range(3):
            t = persist.tile([128, NB8 + 4], _BF16, tag=f"b8_{db}")
            nc.vector.memset(t[:, 0:2], PAD)
            nc.vector.memset(t[:, NB8 + 2 : NB8 + 4], PAD)
            b8[db] = t

        ws_t = {}    # (db, s) -> ws tile
        zsesc = {}   # s -> merged zsE tile [128, 3*scb]
        pending = []  # deferred output copies [(dram_ap, tile_ap)]

        def out3(dram, s, tile3):
            base, scb = SCO[s], SCS[s]
            dst = dram.ap().rearrange("d p c -> p d c")[:, :, base : base + scb]
            return (dst, tile3[:].rearrange("p (d c) -> p d c", d=3))

        def phase_load_tree(db, s):
            """Load chunk, 8->1 min tree into b8[db] cols [2+base, 2+base+scb)."""
            base, scb = SCO[s], SCS[s]
            x = xp.tile([128, 592 * 8], _BF16, tag="x")
            xv = x[:, : scb * 8]
            nc.sync.dma_start(xv, xs.ap()[db, :, base * 8 : (base + scb) * 8])
            xf = xv
            t4 = trp.tile([128, 592 * 4], _BF16, tag="t4")
            nc.vector.tensor_tensor(
                t4[:, : scb * 4].rearrange("p (a b) -> p a b", b=4),
                bass.AP(xf.tensor, xf.offset, [list(xf.ap[0]), [8, scb], [1, 4]]),
                bass.AP(xf.tensor, xf.offset + 4, [list(xf.ap[0]), [8, scb], [1, 4]]),
                mybir.AluOpType.min)
            t4f = t4[:, : scb * 4]
            t2 = trp.tile([128, 592 * 2], _BF16, tag="t2")
            nc.vector.tensor_tensor(
                t2[:, : scb * 2].rearrange("p (a b) -> p a b", b=2),
                bass.AP(t4f.tensor, t4f.offset, [list(t4f.ap[0]), [4, scb], [1, 2]]),
                bass.AP(t4f.tensor, t4f.offset + 2, [list(t4f.ap[0]), [4, scb], [1, 2]]),
                mybir.AluOpType.min)
            t2f = t2[:, : scb * 2]
            nc.vector.tensor_tensor(
                b8[db][:, 2 + base : 2 + base + scb],
                bass.AP(t2f.tensor, t2f.offset, [list(t2f.ap[0]), [2, scb]]),
                bass.AP(t2f.tensor, t2f.offset + 1, [list(t2f.ap[0]), [2, scb]]),
                mybir.AluOpType.min)
            pending.append((b8_d.ap()[db, :, base : base + scb],
                            b8[db][:, 2 + base : 2 + base + scb]))

        p2_t = {}
        wp_t = {}

        def phase_p2(db, s):
            """P2t col j <-> P2 index base-1+j = min(b8[base+j], b8[base+j+1]).
            Needs b8 cols up to base+scb+3 -> first 2 blocks of chunk s+1 or pad."""
            base, scb = SCO[s], SCS[s]
            bt = b8[db]
            p2 = tabp.tile([128, 592 + 2], _BF16, tag=f"p2_{db}")
            nc.vector.tensor_tensor(
                p2[:, : scb + 2], bt[:, base : base + scb + 2],
                bt[:, base + 1 : base + scb + 3], mybir.AluOpType.min)
            p2_t[(db, s)] = p2

        def phase_wp(db, s):
            scb = SCS[s]
            wp = tabp.tile([128, 592 + 2], _BF16, tag=f"wp_{db}")
            nc.scalar.activation(wp[:, : scb + 2], p2_t[(db, s)][:, : scb + 2],
                                 mybir.ActivationFunctionType.Exp,
                                 bias=bexp[:], scale=-C_LN)
            wp_t[(db, s)] = wp
            p2_t.pop((db, s), None)

        def phase_ws(db, s):
            """ws[2mr+b] = wp[2mr+2] + wp[2mr+1+2b]   (cols of wp tile)."""
            scb = SCS[s]
            ws = wsp.tile([128, 592], _BF16, tag=f"ws{db}")
            wpf = wp_t[(db, s)][:]
            in0 = bass.AP(wpf.tensor, wpf.offset + 2,
                          [list(wpf.ap[0]), [2, scb // 2], [0, 2]])
            in1 = bass.AP(wpf.tensor, wpf.offset + 1,
                          [list(wpf.ap[0]), [2, scb // 2], [2, 2]])
            eng = nc.gpsimd if POOL_WS else nc.vector
            eng.tensor_tensor(ws[:, : scb].rearrange("p (a b) -> p a b", b=2),
                              in0, in1, mybir.AluOpType.add)
            ws_t[(db, s)] = ws
            wp_t.pop((db, s), None)

        def phase_matmul(db, s):
            """Adjacency matmul -> zsE (bf16, incl. floor) for (db, s)."""
            scb = SCS[s]
            if db == 0:
                zsem = zsep.tile([128, 3 * scb], _BF16, tag=f"zsem{scb}")
                zsesc[s] = zsem
            zse = zsesc[s]
            for c in range(scb // MMW):
                zp = psp.tile([128, MMW], _F32, tag=f"zp{c}")
                for sb in range(3):
                    nc.tensor.matmul(
                        zp[:], wt[(sb, db)][:],
                        ws_t[(sb, s)][:, MMW * c : MMW * (c + 1)],
                        start=(sb == 0), stop=(sb == 2))
                nc.scalar.activation(
                    zse[:, scb * db + MMW * c : scb * db + MMW * (c + 1)],
                    zp[:], mybir.ActivationFunctionType.Copy,
                    bias=E_FLOOR, scale=1.0)
            if db == 2:
                nc.scalar.dma_start(*out3(zs_d, s, zse))

        for s in range(NSC + 1):
            # flush output copies whose data was produced last iteration;
            # they never block the loads that follow on the SP stream
            flush, pending = pending, []
            for dram_ap, tile_ap in flush:
                nc.sync.dma_start(dram_ap, tile_ap)
            if s < NSC:
                for db in range(3):
                    phase_load_tree(db, s)
                    if s == 0 and db == 0:
                        load_wadj()
            if s >= 1:
                for db in range(3):
                    phase_p2(db, s - 1)
                for db in range(3):
                    phase_wp(db, s - 1)
                for db in range(3):
                    phase_ws(db, s - 1)
                for db in range(3):
                    phase_matmul(db, s - 1)
                for db in range(3):
                    ws_t.pop((db, s - 1), None)
        for dram_ap, tile_ap in pending:
            nc.sync.dma_start(dram_ap, tile_ap)

    nc.compile()
    return nc


# ------------------------ host side ------------------------

def _adjacency(channel_locations):
    locs = np.asarray(channel_locations, np.float32)
    d2 = ((locs[:, None, :] - locs[None, :, :]) ** 2).sum(-1, dtype=np.float32)
    return np.sqrt(d2.astype(np.float32)) <= np.float32(RADIUS)


def _const_inputs(adj):
    adj_f = adj.astype(np.float32)
    a = adj_f.reshape(3, 128, 3, 128).transpose(0, 2, 1, 3)
    return np.ascontiguousarray(a).astype(ml_dtypes.bfloat16)


def _nbr_table(adj):
    deg = adj.sum(0)
    dmax = int(deg.max())
    nbr = np.zeros((M, dmax), np.int32)
    for m in range(M):
        js = np.flatnonzero(adj[:, m])
        nbr[m, : len(js)] = js
        nbr[m, len(js):] = js[0] if len(js) else m
    return nbr


def _resolve_core(flag, c8_inv, bnd, traces, nbr, start, g0, guard_h):
    """One core: flagged blocks -> exact detections (times, chans) sorted.
    flag/c8_inv/bnd: [384, NB8]. start: global row of local col 0;
    g0: global row of interior start."""
    ch, k8 = np.nonzero(flag)
    if ch.size == 0:
        return np.empty(0, np.int64), np.empty(0, np.int64)
    tg = (start + k8 * 8)[:, None] + np.arange(8)[None, :]    # [P, 8]
    xv = -traces[tg, ch[:, None]]                              # [P, 8]
    keep = xv >= np.maximum(THR, (bnd[ch, k8] - guard_h)[:, None])
    keep &= (tg >= max(MARGIN, g0)) & (tg < min(N - MARGIN, g0 + INT))
    pi, ji = np.nonzero(keep)
    if pi.size == 0:
        return np.empty(0, np.int64), np.empty(0, np.int64)
    mm = ch[pi].astype(np.int64)
    k8c = k8[pi]
    tgc = tg[pi, ji]
    xvc = xv[pi, ji]

    cwn = nbr[mm]                                   # [P, D]
    c8n = c8_inv[cwn, k8c[:, None]]                 # [P, D]
    m1 = c8n.max(1)
    sure = xvc >= m1 + SLACK_SURE
    ok = sure.copy()
    amb = np.flatnonzero(~sure)
    if amb.size:
        tga = tgc[amb]
        xva = xvc[amb]
        cwa = cwn[amb]
        live = c8n[amb] >= xva[:, None] - SLACK_SURE
        pi2, di2 = np.nonzero(live)
        bad = np.zeros(amb.size, bool)
        if pi2.size:
            tt = tga[pi2]
            jj = cwa[pi2, di2]
            t0 = np.maximum(tt - TR, 0)
            t1 = np.minimum(tt + TR, N - 1)
            tw = t0[:, None] + np.arange(2 * TR + 1)[None, :]
            np.minimum(tw, t1[:, None], out=tw)
            g = traces[tw, jj[:, None]]
            svp = -(g.min(1))
            veto = svp > xva[pi2]
            bad = np.bincount(pi2, weights=veto.astype(np.float64),
                              minlength=amb.size) > 0
        ok[amb] = ~bad
    mm, tgc = mm[ok], tgc[ok]
    o = np.lexsort((mm, tgc))
    return tgc[o].astype(np.int64), mm[o]


_PROGRAM_CACHE = {}


def kernel(traces, channel_locations):
    traces = np.ascontiguousarray(np.asarray(traces, np.float32))
    adj = _adjacency(channel_locations)
    wa = _const_inputs(adj)
    nbr = _nbr_table(adj)
    degmax = int(adj.sum(1).max())
    slack_deg = float(np.log(2.0 * degmax + 1.0)) / C_LN
    guard_h = slack_deg + D_EPS
    guard_f = guard_h + 0.014   # + bf16 rounding of b8 vs raw samples

    x_bf = traces.astype(ml_dtypes.bfloat16)

    if "full" not in _PROGRAM_CACHE:
        _PROGRAM_CACHE["full"] = build_program()
    nc = _PROGRAM_CACHE["full"]

    starts = [min(max(c * INT - TR, 0), N - T_LOC) for c in range(NCORES)]
    in_maps = [{
        "xs": np.ascontiguousarray(
            x_bf[starts[c] : starts[c] + T_LOC].T).reshape(3, 128, T_LOC),
        "wadj": wa,
    } for c in range(NCORES)]
    try:
        res = run_bass_kernel_spmd(nc, in_maps, list(range(NCORES)))
    except Exception:
        time.sleep(2.0)
        res = run_bass_kernel_spmd(nc, in_maps, list(range(NCORES)))
    results = res.results

    all_t, all_c = [], []
    for c in range(NCORES):
        r = results[c]
        b8_inv = -np.asarray(r["b8"]).astype(np.float32).reshape(384, NB8)
        # c8 = 5-block sliding cover max of b8 (same values the device would
        # have produced: bf16 max is exact)
        b8p = np.full((384, NB8 + 4), -np.float32(1e30), np.float32)
        b8p[:, 2 : NB8 + 2] = b8_inv
        c8_inv = np.maximum.reduce(
            [b8p[:, i : i + NB8] for i in range(5)])
        zse = np.asarray(r["zse"]).astype(np.float32).reshape(384, NB8)
        bnd = np.log(zse) / C_LN + B0
        flag = b8_inv >= np.maximum(THR - 0.02, bnd - guard_f)
        t_, c_ = _resolve_core(flag, c8_inv, bnd, traces, nbr,
                               starts[c], c * INT, guard_h)
        all_t.append(t_)
        all_c.append(c_)

    times = np.concatenate(all_t) if all_t else np.empty(0, np.int64)
    chans = np.concatenate(all_c) if all_c else np.empty(0, np.int64)
    times, chans = times[:MAX_DET], chans[:MAX_DET]
    out_t = np.full(MAX_DET, -1, np.int64)
    out_c = np.full(MAX_DET, -1, np.int32)
    out_t[: times.size] = times
    out_c[: chans.size] = chans
    return out_t, out_c


# revision 5
# speedup vs baseline: 1.0671x; 1.0671x over previous
"""Trainium2 Bass kernel for nn_DetectSpikes (spatiotemporal NMS spike detection).

kernel(traces [150000,384] f32, channel_locations [384,2] f32) ->
(times int64 [100000], chans int32 [100000]) matching the reference exactly.

Detection rule (x_inv = -traces): (n, m) is a detection iff x_inv >= 3.0,
time margin, and x_inv >= max over adj(m) x [n-15, n+15] (ties pass).

Device (8 cores, time-sharded with halo, SPMD). All device math runs in the
min-domain (raw traces, bf16; block MIN equals x_inv block max, negated):
  - Host pre-transposes traces to [chan, time] bf16: no PE transpose, half
    the HBM traffic of f32.
  - 8-sample block minima B8 per channel via a 3-step min tree (DVE).
  - P2 = pairwise min of B8; c8 = 5-block cover min (bf16 cover table for
    the host sure-check).
  - LSE screen at block resolution: wp = exp(-C*(P2+B0)) (ACT),
    ws = two-offset 16-sample grid sum, zs = adjacency matmul (PE, bf16),
    zsE = zs + floor (ACT copy from PSUM, bf16 out).
  Outputs per block: B8, c8, zsE (all bf16) -- 3/8 the input volume.
Host: flags the ~1% of blocks with B8 >= ln(zsE)/C + B0 - guard, screens
  their 8 samples against the same bound (guards cover every bf16/exp
  error), classifies sure vs ambiguous via the c8 cover table, and
  resolves the ambiguous ones exactly from raw f32 traces. Output exact.
"""

import time

import numpy as np
import ml_dtypes

import concourse.bass as bass
import concourse.tile as tile
from concourse import bacc, mybir
from concourse.bass_utils import run_bass_kernel_spmd

# ---- problem constants ----
N, M = 150000, 384
TR = 15
THR = 3.0
MARGIN = 20
RADIUS = 100.0
MAX_DET = 100000
NCORES = 8
INT = N // NCORES             # 18750

T_LOC = 18944                 # 8*2368 >= INT + 2*TR, multiple of 32
NB8 = T_LOC // 8              # 2368
SCS = [296, 592, 592, 444, 296, 148]  # super-chunk sizes (blocks); sum = NB8
SCO = [0, 296, 888, 1480, 1924, 2220]  # block offsets
NSC = len(SCS)
MMW = 148                     # matmul chunk (one PSUM bank)

PAD = 240.0                   # min-domain pad (= -240 in x_inv domain)
C_LN = float(32.0 * np.log(2.0))
B0 = 5.4
THR_FLOOR = 2.93
E_FLOOR = float(np.exp(C_LN * (THR_FLOOR - B0)))
SLACK_SURE = 0.02             # host sure-check slack for bf16 rounding
D_EPS = 0.004                 # exp/matmul/bf16-zse error stack (host guard)

POOL_WS = False               # ws add runs on DVE (shorter chain)

_F32 = mybir.dt.float32
_BF16 = mybir.dt.bfloat16
_U8 = mybir.dt.uint8


def build_program():
    nc = bacc.Bacc(
        "TRN2", target_bir_lowering=False, debug=False, enable_asserts=False,
        num_devices=NCORES,
    )
    xs = nc.dram_tensor("xs", [3, 128, T_LOC], _BF16, kind="ExternalInput")
    wadj = nc.dram_tensor("wadj", [3, 3, 128, 128], _BF16, kind="ExternalInput")
    zs_d = nc.dram_tensor("zse", [3, 128, NB8], _BF16, kind="ExternalOutput")
    b8_d = nc.dram_tensor("b8", [3, 128, NB8], _BF16, kind="ExternalOutput")

    BEXP = float(np.float32(-C_LN * B0))

    from contextlib import ExitStack
    with tile.TileContext(nc) as tc, ExitStack() as ctx:
        consts = ctx.enter_context(tc.tile_pool(name="consts", bufs=1))
        persist = ctx.enter_context(tc.tile_pool(name="persist", bufs=1))
        xp = ctx.enter_context(tc.tile_pool(name="xp", bufs=5))
        trp = ctx.enter_context(tc.tile_pool(name="trp", bufs=2))
        tabp = ctx.enter_context(tc.tile_pool(name="tabp", bufs=2))
        wsp = ctx.enter_context(tc.tile_pool(name="wsp", bufs=6))
        zsep = ctx.enter_context(tc.tile_pool(name="zsep", bufs=2))
        outp = ctx.enter_context(tc.tile_pool(name="outp", bufs=3))
        psp = ctx.enter_context(tc.tile_pool(name="psp", bufs=2, space="PSUM"))

        # adjacency blocks [c_sb partitions, c_db]
        wa_all = consts.tile([128, 9 * 128], _BF16, tag="wa_all")
        wt = {}
        for sb in range(3):
            for db in range(3):
                wt[(sb, db)] = wa_all[:, 128 * (3 * sb + db) : 128 * (3 * sb + db) + 128]

        def load_wadj():
            for sb in range(3):
                for db in range(3):
                    nc.scalar.dma_start(wt[(sb, db)], wadj.ap()[sb, db, :, :])
        bexp = consts.tile([128, 1], _F32, tag="bexp")
        nc.vector.memset(bexp[:], BEXP)

        # B8 block minima, padded by 2 cols each side (col j <-> block j-2)
        b8 = {}
        for db in range(3):
            t = persist.tile([128, NB8 + 4], _BF16, tag=f"b8_{db}")
            nc.vector.memset(t[:, 0:2], PAD)
            nc.vector.memset(t[:, NB8 + 2 : NB8 + 4], PAD)
            b8[db] = t

        ws_t = {}    # (db, s) -> ws tile
        zsesc = {}   # s -> merged zsE tile [128, 3*scb]
        pending = []  # deferred output copies [(dram_ap, tile_ap)]

        def out3(dram, s, tile3):
            base, scb = SCO[s], SCS[s]
            dst = dram.ap().rearrange("d p c -> p d c")[:, :, base : base + scb]
            return (dst, tile3[:].rearrange("p (d c) -> p d c", d=3))

        def phase_load_tree(db, s):
            """Load chunk, 8->1 min tree into b8[db] cols [2+base, 2+base+scb)."""
            base, scb = SCO[s], SCS[s]
            x = xp.tile([128, 592 * 8], _BF16, tag="x")
            xv = x[:, : scb * 8]
            nc.sync.dma_start(xv, xs.ap()[db, :, base * 8 : (base + scb) * 8])
            xf = xv
            t4 = trp.tile([128, 592 * 4], _BF16, tag="t4")
            nc.vector.tensor_tensor(
                t4[:, : scb * 4].rearrange("p (a b) -> p a b", b=4),
                bass.AP(xf.tensor, xf.offset, [list(xf.ap[0]), [8, scb], [1, 4]]),
                bass.AP(xf.tensor, xf.offset + 4, [list(xf.ap[0]), [8, scb], [1, 4]]),
                mybir.AluOpType.min)
            t4f = t4[:, : scb * 4]
            t2 = trp.tile([128, 592 * 2], _BF16, tag="t2")
            nc.vector.tensor_tensor(
                t2[:, : scb * 2].rearrange("p (a b) -> p a b", b=2),
                bass.AP(t4f.tensor, t4f.offset, [list(t4f.ap[0]), [4, scb], [1, 2]]),
                bass.AP(t4f.tensor, t4f.offset + 2, [list(t4f.ap[0]), [4, scb], [1, 2]]),
                mybir.AluOpType.min)
            t2f = t2[:, : scb * 2]
            nc.vector.tensor_tensor(
                b8[db][:, 2 + base : 2 + base + scb],
                bass.AP(t2f.tensor, t2f.offset, [list(t2f.ap[0]), [2, scb]]),
                bass.AP(t2f.tensor, t2f.offset + 1, [list(t2f.ap[0]), [2, scb]]),
                mybir.AluOpType.min)
            pending.append((b8_d.ap()[db, :, base : base + scb],
                            b8[db][:, 2 + base : 2 + base + scb]))

        p2_t = {}
        wp_t = {}

        def phase_p2(db, s):
            """P2t col j <-> P2 index base-1+j = min(b8[base+j], b8[base+j+1]).
            Needs b8 cols up to base+scb+3 -> first 2 blocks of chunk s+1 or pad."""
            base, scb = SCO[s], SCS[s]
            bt = b8[db]
            p2 = tabp.tile([128, 592 + 2], _BF16, tag=f"p2_{db}")
            nc.vector.tensor_tensor(
                p2[:, : scb + 2], bt[:, base : base + scb + 2],
                bt[:, base + 1 : base + scb + 3], mybir.AluOpType.min)
            p2_t[(db, s)] = p2

        def phase_wp(db, s):
            scb = SCS[s]
            wp = tabp.tile([128, 592 + 2], _BF16, tag=f"wp_{db}")
            nc.scalar.activation(wp[:, : scb + 2], p2_t[(db, s)][:, : scb + 2],
                                 mybir.ActivationFunctionType.Exp,
                                 bias=bexp[:], scale=-C_LN)
            wp_t[(db, s)] = wp
            p2_t.pop((db, s), None)


        def phase_matmul(db, s):
            """Adjacency matmul on wp directly (two strided rhs views per
            source block accumulate the two grid covers) -> zsE."""
            scb = SCS[s]
            if db == 0:
                zsem = zsep.tile([128, 3 * scb], _BF16, tag=f"zsem{scb}")
                zsesc[s] = zsem
            zse = zsesc[s]
            for c in range(scb // MMW):
                zp = psp.tile([128, MMW], _F32, tag=f"zp{c}")
                zpo = zp[:].rearrange("p (a b) -> p a b", b=2)
                for sb in range(3):
                    wpf = wp_t[(sb, s)][:]
                    rhs0 = bass.AP(wpf.tensor, wpf.offset + MMW * c + 2,
                                   [list(wpf.ap[0]), [2, MMW // 2], [0, 2]])
                    rhs1 = bass.AP(wpf.tensor, wpf.offset + MMW * c + 1,
                                   [list(wpf.ap[0]), [2, MMW // 2], [2, 2]])
                    nc.tensor.matmul(zpo, wt[(sb, db)][:], rhs0,
                                     start=(sb == 0), stop=False)
                    nc.tensor.matmul(zpo, wt[(sb, db)][:], rhs1,
                                     start=False, stop=(sb == 2))
                nc.scalar.activation(
                    zse[:, scb * db + MMW * c : scb * db + MMW * (c + 1)],
                    zp[:], mybir.ActivationFunctionType.Copy,
                    bias=E_FLOOR, scale=1.0)
            if db == 2:
                nc.scalar.dma_start(*out3(zs_d, s, zse))

        for s in range(NSC + 1):
            # flush output copies whose data was produced last iteration;
            # they never block the loads that follow on the SP stream
            flush, pending = pending, []
            for dram_ap, tile_ap in flush:
                nc.sync.dma_start(dram_ap, tile_ap)
            if s < NSC:
                for db in range(3):
                    phase_load_tree(db, s)
                    if s == 0 and db == 0:
                        load_wadj()
            if s >= 1:
                for db in range(3):
                    phase_p2(db, s - 1)
                for db in range(3):
                    phase_wp(db, s - 1)
                for db in range(3):
                    phase_matmul(db, s - 1)
                for db in range(3):
                    wp_t.pop((db, s - 1), None)
        for dram_ap, tile_ap in pending:
            nc.sync.dma_start(dram_ap, tile_ap)

    nc.compile()
    return nc


# ------------------------ host side ------------------------

def _adjacency(channel_locations):
    locs = np.asarray(channel_locations, np.float32)
    d2 = ((locs[:, None, :] - locs[None, :, :]) ** 2).sum(-1, dtype=np.float32)
    return np.sqrt(d2.astype(np.float32)) <= np.float32(RADIUS)


def _const_inputs(adj):
    adj_f = adj.astype(np.float32)
    a = adj_f.reshape(3, 128, 3, 128).transpose(0, 2, 1, 3)
    return np.ascontiguousarray(a).astype(ml_dtypes.bfloat16)


def _nbr_table(adj):
    deg = adj.sum(0)
    dmax = int(deg.max())
    nbr = np.zeros((M, dmax), np.int32)
    for m in range(M):
        js = np.flatnonzero(adj[:, m])
        nbr[m, : len(js)] = js
        nbr[m, len(js):] = js[0] if len(js) else m
    return nbr


def _resolve_core(flag, c8_inv, bnd, traces, nbr, start, g0, guard_h):
    """One core: flagged blocks -> exact detections (times, chans) sorted.
    flag/c8_inv/bnd: [384, NB8]. start: global row of local col 0;
    g0: global row of interior start."""
    ch, k8 = np.nonzero(flag)
    if ch.size == 0:
        return np.empty(0, np.int64), np.empty(0, np.int64)
    tg = (start + k8 * 8)[:, None] + np.arange(8)[None, :]    # [P, 8]
    xv = -traces[tg, ch[:, None]]                              # [P, 8]
    keep = xv >= np.maximum(THR, (bnd[ch, k8] - guard_h)[:, None])
    keep &= (tg >= max(MARGIN, g0)) & (tg < min(N - MARGIN, g0 + INT))
    pi, ji = np.nonzero(keep)
    if pi.size == 0:
        return np.empty(0, np.int64), np.empty(0, np.int64)
    mm = ch[pi].astype(np.int64)
    k8c = k8[pi]
    tgc = tg[pi, ji]
    xvc = xv[pi, ji]

    cwn = nbr[mm]                                   # [P, D]
    c8n = c8_inv[cwn, k8c[:, None]]                 # [P, D]
    m1 = c8n.max(1)
    sure = xvc >= m1 + SLACK_SURE
    ok = sure.copy()
    amb = np.flatnonzero(~sure)
    if amb.size:
        tga = tgc[amb]
        xva = xvc[amb]
        cwa = cwn[amb]
        live = c8n[amb] >= xva[:, None] - SLACK_SURE
        pi2, di2 = np.nonzero(live)
        bad = np.zeros(amb.size, bool)
        if pi2.size:
            tt = tga[pi2]
            jj = cwa[pi2, di2]
            t0 = np.maximum(tt - TR, 0)
            t1 = np.minimum(tt + TR, N - 1)
            tw = t0[:, None] + np.arange(2 * TR + 1)[None, :]
            np.minimum(tw, t1[:, None], out=tw)
            g = traces[tw, jj[:, None]]
            svp = -(g.min(1))
            veto = svp > xva[pi2]
            bad = np.bincount(pi2, weights=veto.astype(np.float64),
                              minlength=amb.size) > 0
        ok[amb] = ~bad
    mm, tgc = mm[ok], tgc[ok]
    o = np.lexsort((mm, tgc))
    return tgc[o].astype(np.int64), mm[o]


_PROGRAM_CACHE = {}


def kernel(traces, channel_locations):
    traces = np.ascontiguousarray(np.asarray(traces, np.float32))
    adj = _adjacency(channel_locations)
    wa = _const_inputs(adj)
    nbr = _nbr_table(adj)
    degmax = int(adj.sum(1).max())
    slack_deg = float(np.log(2.0 * degmax + 1.0)) / C_LN
    guard_h = slack_deg + D_EPS
    guard_f = guard_h + 0.014   # + bf16 rounding of b8 vs raw samples

    x_bf = traces.astype(ml_dtypes.bfloat16)

    if "full" not in _PROGRAM_CACHE:
        _PROGRAM_CACHE["full"] = build_program()
    nc = _PROGRAM_CACHE["full"]

    starts = [min(max(c * INT - TR, 0), N - T_LOC) for c in range(NCORES)]
    in_maps = [{
        "xs": np.ascontiguousarray(
            x_bf[starts[c] : starts[c] + T_LOC].T).reshape(3, 128, T_LOC),
        "wadj": wa,
    } for c in range(NCORES)]
    try:
        res = run_bass_kernel_spmd(nc, in_maps, list(range(NCORES)))
    except Exception:
        time.sleep(2.0)
        res = run_bass_kernel_spmd(nc, in_maps, list(range(NCORES)))
    results = res.results

    all_t, all_c = [], []
    for c in range(NCORES):
        r = results[c]
        b8_inv = -np.asarray(r["b8"]).astype(np.float32).reshape(384, NB8)
        # c8 = 5-block sliding cover max of b8 (same values the device would
        # have produced: bf16 max is exact)
        b8p = np.full((384, NB8 + 4), -np.float32(1e30), np.float32)
        b8p[:, 2 : NB8 + 2] = b8_inv
        c8_inv = np.maximum.reduce(
            [b8p[:, i : i + NB8] for i in range(5)])
        zse = np.asarray(r["zse"]).astype(np.float32).reshape(384, NB8)
        bnd = np.log(zse) / C_LN + B0
        flag = b8_inv >= np.maximum(THR - 0.02, bnd - guard_f)
        t_, c_ = _resolve_core(flag, c8_inv, bnd, traces, nbr,
                               starts[c], c * INT, guard_h)
        all_t.append(t_)
        all_c.append(c_)

    times = np.concatenate(all_t) if all_t else np.empty(0, np.int64)
    chans = np.concatenate(all_c) if all_c else np.empty(0, np.int64)
    times, chans = times[:MAX_DET], chans[:MAX_DET]
    out_t = np.full(MAX_DET, -1, np.int64)
    out_c = np.full(MAX_DET, -1, np.int32)
    out_t[: times.size] = times
    out_c[: chans.size] = chans
    return out_t, out_c


# revision 6
# speedup vs baseline: 1.1198x; 1.0493x over previous
"""Trainium2 Bass kernel for nn_DetectSpikes (spatiotemporal NMS spike detection).

kernel(traces [150000,384] f32, channel_locations [384,2] f32) ->
(times int64 [100000], chans int32 [100000]) matching the reference exactly.

Detection rule (x_inv = -traces): (n, m) is a detection iff x_inv >= 3.0,
time margin, and x_inv >= max over adj(m) x [n-15, n+15] (ties pass).

Device (8 cores, time-sharded with halo, SPMD). All math in min-domain
(raw traces, bf16; block MIN equals x_inv block max, negated):
  - Host pre-transposes traces to [chan, time] bf16: no PE transpose, half
    the HBM traffic of f32.
  - 8-sample block minima B8 per channel via a 3-step min tree (DVE).
    Chunks overlap by one block so each chunk's chain is self-contained.
  - P2 = pairwise min of B8 (clamped at -1.7 by construction of exp arg),
    wp = exp(-C*(P2+B0)) (ACT),
    zs = adjacency matmul over wp with two strided rhs views per source
    (the two 16-sample grid covers) accumulated in PSUM (PE, bf16),
    zsE = zs + floor (ACT copy from PSUM, bf16 out).
  Outputs per block: B8 and zsE (bf16) -- 1/4 the input volume.
Host: flags the ~1% of blocks with B8 >= ln(zsE)/C + B0 - guard, screens
  their 8 samples against the same bound (guards cover every bf16/exp
  error), derives the 5-block cover table c8 from B8, classifies sure vs
  ambiguous, resolves ambiguous exactly from raw f32 traces. Output exact.
"""

import time

import numpy as np
import ml_dtypes

import concourse.bass as bass
import concourse.tile as tile
from concourse import bacc, mybir
from concourse.bass_utils import run_bass_kernel_spmd

# ---- problem constants ----
N, M = 150000, 384
TR = 15
THR = 3.0
MARGIN = 20
RADIUS = 100.0
MAX_DET = 100000
NCORES = 8
INT = N // NCORES             # 18750

T_LOC = 18944                 # 8*2368 >= INT + 2*TR, multiple of 32
NB8 = T_LOC // 8              # 2368
SCS = [296, 592, 592, 444, 296, 148]  # super-chunk sizes (blocks); sum = NB8
SCO = [0, 296, 888, 1480, 1924, 2220]  # block offsets
NSC = len(SCS)
MMW = 148                     # matmul chunk (one PSUM bank)

PAD = 240.0                   # min-domain pad (= -240 in x_inv domain)
C_LN = float(32.0 * np.log(2.0))
B0 = 5.4
THR_FLOOR = 2.93
E_FLOOR = float(np.exp(C_LN * (THR_FLOOR - B0)))
SLACK_SURE = 0.02             # host sure-check slack for bf16 rounding
D_EPS = 0.004                 # exp/matmul/bf16-zse error stack (host guard)

_F32 = mybir.dt.float32
_BF16 = mybir.dt.bfloat16
_U8 = mybir.dt.uint8


def build_program():
    nc = bacc.Bacc(
        "TRN2", target_bir_lowering=False, debug=False, enable_asserts=False,
        num_devices=NCORES,
    )
    xs = nc.dram_tensor("xs", [3, 128, T_LOC], _BF16, kind="ExternalInput")
    wadj = nc.dram_tensor("wadj", [3, 3, 128, 128], _BF16, kind="ExternalInput")
    zs_d = nc.dram_tensor("zse", [3, 128, NB8], _BF16, kind="ExternalOutput")
    b8_d = nc.dram_tensor("b8", [3, 128, NB8], _BF16, kind="ExternalOutput")

    BEXP = float(np.float32(-C_LN * B0))

    from contextlib import ExitStack
    with tile.TileContext(nc) as tc, ExitStack() as ctx:
        consts = ctx.enter_context(tc.tile_pool(name="consts", bufs=1))
        persist = ctx.enter_context(tc.tile_pool(name="persist", bufs=1))
        xp = ctx.enter_context(tc.tile_pool(name="xp", bufs=7))
        trp = ctx.enter_context(tc.tile_pool(name="trp", bufs=2))
        tabp = ctx.enter_context(tc.tile_pool(name="tabp", bufs=2))
        zsep = ctx.enter_context(tc.tile_pool(name="zsep", bufs=2))
        psp = ctx.enter_context(tc.tile_pool(name="psp", bufs=2, space="PSUM"))

        wt = {}
        for sb in range(3):
            for db in range(3):
                wtile = consts.tile([128, 128], _BF16, tag=f"wa{sb}{db}")
                wt[(sb, db)] = wtile

        def load_wadj():
            for sb in range(3):
                for db in range(3):
                    nc.scalar.dma_start(wt[(sb, db)][:], wadj.ap()[sb, db, :, :])

        bexp = consts.tile([128, 1], _F32, tag="bexp")
        nc.vector.memset(bexp[:], BEXP)

        # B8 block minima, padded 2 cols each side (tile col j <-> block j-2)
        b8 = {}
        for db in range(3):
            t = persist.tile([128, NB8 + 4], _BF16, tag=f"b8_{db}")
            nc.vector.memset(t[:, 0:2], PAD)
            nc.vector.memset(t[:, NB8 + 2 : NB8 + 4], PAD)
            b8[db] = t

        wp_t = {}
        zsesc = {}
        pending = []  # deferred SP output copies

        def phase_load_tree(db, s):
            """Load blocks [t0, t1) (chunk +- 1 block), min-tree into b8."""
            base, scb = SCO[s], SCS[s]
            t0 = max(base - 1, 0)
            t1 = min(base + scb + 1, NB8)
            nb = t1 - t0
            x = xp.tile([128, 594 * 8], _BF16, tag="x")
            xv = x[:, : nb * 8]
            nc.sync.dma_start(xv, xs.ap()[db, :, t0 * 8 : t1 * 8])
            xf = xv
            t4 = trp.tile([128, 594 * 4], _BF16, tag="t4")
            nc.vector.tensor_tensor(
                t4[:, : nb * 4].rearrange("p (a b) -> p a b", b=4),
                bass.AP(xf.tensor, xf.offset, [list(xf.ap[0]), [8, nb], [1, 4]]),
                bass.AP(xf.tensor, xf.offset + 4, [list(xf.ap[0]), [8, nb], [1, 4]]),
                mybir.AluOpType.min)
            t4f = t4[:, : nb * 4]
            t2 = trp.tile([128, 594 * 2], _BF16, tag="t2")
            nc.vector.tensor_tensor(
                t2[:, : nb * 2].rearrange("p (a b) -> p a b", b=2),
                bass.AP(t4f.tensor, t4f.offset, [list(t4f.ap[0]), [4, nb], [1, 2]]),
                bass.AP(t4f.tensor, t4f.offset + 2, [list(t4f.ap[0]), [4, nb], [1, 2]]),
                mybir.AluOpType.min)
            t2f = t2[:, : nb * 2]
            nc.vector.tensor_tensor(
                b8[db][:, 2 + t0 : 2 + t1],
                bass.AP(t2f.tensor, t2f.offset, [list(t2f.ap[0]), [2, nb]]),
                bass.AP(t2f.tensor, t2f.offset + 1, [list(t2f.ap[0]), [2, nb]]),
                mybir.AluOpType.min)
            pending.append((b8_d.ap()[db, :, base : base + scb],
                            b8[db][:, 2 + base : 2 + base + scb]))

        def phase_p2(db, s):
            """P2t[jj] = P2[base+jj] = min(b8 cols base+jj+1, base+jj+2),
            jj in [0, scb+1). Self-contained: tree(s) covers all blocks."""
            base, scb = SCO[s], SCS[s]
            bt = b8[db]
            p2 = tabp.tile([128, 592 + 1], _BF16, tag=f"p2_{db}")
            nc.vector.tensor_tensor(
                p2[:, : scb + 1], bt[:, base + 1 : base + scb + 2],
                bt[:, base + 2 : base + scb + 3], mybir.AluOpType.min)
            wp = tabp.tile([128, 592 + 1], _BF16, tag=f"wp_{db}")
            nc.scalar.activation(wp[:, : scb + 1], p2[:, : scb + 1],
                                 mybir.ActivationFunctionType.Exp,
                                 bias=bexp[:], scale=-C_LN)
            wp_t[(db, s)] = wp

        def phase_matmul(db, s):
            """zs[k8=base+2mr+b] = sum_adj (wp[tile 2mr+1] + wp[tile 2mr+2b]),
            two strided rhs views per source block, accumulated in PSUM."""
            base, scb = SCO[s], SCS[s]
            if db == 0:
                zsem = zsep.tile([128, 3 * scb], _BF16, tag=f"zsem{scb}")
                zsesc[s] = zsem
            zse = zsesc[s]
            for c in range(scb // MMW):
                zp = psp.tile([128, MMW], _F32, tag=f"zp{c % 2}")
                zpo = zp[:].rearrange("p (a b) -> p a b", b=2)
                for sb in range(3):
                    wpf = wp_t[(sb, s)][:]
                    rhs0 = bass.AP(wpf.tensor, wpf.offset + MMW * c + 1,
                                   [list(wpf.ap[0]), [2, MMW // 2], [0, 2]])
                    rhs1 = bass.AP(wpf.tensor, wpf.offset + MMW * c,
                                   [list(wpf.ap[0]), [2, MMW // 2], [2, 2]])
                    nc.tensor.matmul(zpo, wt[(sb, db)][:], rhs0,
                                     start=(sb == 0), stop=False)
                    nc.tensor.matmul(zpo, wt[(sb, db)][:], rhs1,
                                     start=False, stop=(sb == 2))
                nc.scalar.activation(
                    zse[:, scb * db + MMW * c : scb * db + MMW * (c + 1)],
                    zp[:], mybir.ActivationFunctionType.Copy,
                    bias=E_FLOOR, scale=1.0)
            if db == 2:
                dst = zs_d.ap().rearrange("d p c -> p d c")[:, :, base : base + scb]
                nc.scalar.dma_start(dst, zse[:].rearrange("p (d c) -> p d c", d=3))

        for s in range(NSC):
            flush, pending = pending, []
            for dram_ap, tile_ap in flush:
                nc.sync.dma_start(dram_ap, tile_ap)
            for db in range(3):
                phase_load_tree(db, s)
                if s == 0 and db == 0:
                    load_wadj()
            for db in range(3):
                phase_p2(db, s)
            for db in range(3):
                phase_matmul(db, s)
            for db in range(3):
                wp_t.pop((db, s), None)
        for dram_ap, tile_ap in pending:
            nc.sync.dma_start(dram_ap, tile_ap)

    nc.compile()
    return nc


# ------------------------ host side ------------------------

def _adjacency(channel_locations):
    locs = np.asarray(channel_locations, np.float32)
    d2 = ((locs[:, None, :] - locs[None, :, :]) ** 2).sum(-1, dtype=np.float32)
    return np.sqrt(d2.astype(np.float32)) <= np.float32(RADIUS)


def _const_inputs(adj):
    adj_f = adj.astype(np.float32)
    a = adj_f.reshape(3, 128, 3, 128).transpose(0, 2, 1, 3)
    return np.ascontiguousarray(a).astype(ml_dtypes.bfloat16)


def _nbr_table(adj):
    deg = adj.sum(0)
    dmax = int(deg.max())
    nbr = np.zeros((M, dmax), np.int32)
    for m in range(M):
        js = np.flatnonzero(adj[:, m])
        nbr[m, : len(js)] = js
        nbr[m, len(js):] = js[0] if len(js) else m
    return nbr


def _resolve_core(flag, c8_inv, bnd, traces, nbr, start, g0, guard_h):
    """One core: flagged blocks -> exact detections (times, chans) sorted."""
    ch, k8 = np.nonzero(flag)
    if ch.size == 0:
        return np.empty(0, np.int64), np.empty(0, np.int64)
    tg = (start + k8 * 8)[:, None] + np.arange(8)[None, :]    # [P, 8]
    xv = -traces[tg, ch[:, None]]                              # [P, 8]
    keep = xv >= np.maximum(THR, (bnd[ch, k8] - guard_h)[:, None])
    keep &= (tg >= max(MARGIN, g0)) & (tg < min(N - MARGIN, g0 + INT))
    pi, ji = np.nonzero(keep)
    if pi.size == 0:
        return np.empty(0, np.int64), np.empty(0, np.int64)
    mm = ch[pi].astype(np.int64)
    k8c = k8[pi]
    tgc = tg[pi, ji]
    xvc = xv[pi, ji]

    cwn = nbr[mm]                                   # [P, D]
    c8n = c8_inv[cwn, k8c[:, None]]                 # [P, D]
    m1 = c8n.max(1)
    sure = xvc >= m1 + SLACK_SURE
    ok = sure.copy()
    amb = np.flatnonzero(~sure)
    if amb.size:
        tga = tgc[amb]
        xva = xvc[amb]
        cwa = cwn[amb]
        live = c8n[amb] >= xva[:, None] - SLACK_SURE
        pi2, di2 = np.nonzero(live)
        bad = np.zeros(amb.size, bool)
        if pi2.size:
            tt = tga[pi2]
            jj = cwa[pi2, di2]
            t0 = np.maximum(tt - TR, 0)
            t1 = np.minimum(tt + TR, N - 1)
            tw = t0[:, None] + np.arange(2 * TR + 1)[None, :]
            np.minimum(tw, t1[:, None], out=tw)
            g = traces[tw, jj[:, None]]
            svp = -(g.min(1))
            veto = svp > xva[pi2]
            bad = np.bincount(pi2, weights=veto.astype(np.float64),
                              minlength=amb.size) > 0
        ok[amb] = ~bad
    mm, tgc = mm[ok], tgc[ok]
    o = np.lexsort((mm, tgc))
    return tgc[o].astype(np.int64), mm[o]


_PROGRAM_CACHE = {}


def kernel(traces, channel_locations):
    traces = np.ascontiguousarray(np.asarray(traces, np.float32))
    adj = _adjacency(channel_locations)
    wa = _const_inputs(adj)
    nbr = _nbr_table(adj)
    degmax = int(adj.sum(1).max())
    slack_deg = float(np.log(2.0 * degmax + 1.0)) / C_LN
    guard_h = slack_deg + D_EPS
    guard_f = guard_h + 0.014   # + bf16 rounding of b8 vs raw samples

    x_bf = traces.astype(ml_dtypes.bfloat16)

    if "full" not in _PROGRAM_CACHE:
        _PROGRAM_CACHE["full"] = build_program()
    nc = _PROGRAM_CACHE["full"]

    starts = [min(max(c * INT - TR, 0), N - T_LOC) for c in range(NCORES)]
    in_maps = [{
        "xs": np.ascontiguousarray(
            x_bf[starts[c] : starts[c] + T_LOC].T).reshape(3, 128, T_LOC),
        "wadj": wa,
    } for c in range(NCORES)]
    try:
        res = run_bass_kernel_spmd(nc, in_maps, list(range(NCORES)))
    except Exception:
        time.sleep(2.0)
        res = run_bass_kernel_spmd(nc, in_maps, list(range(NCORES)))
    results = res.results

    all_t, all_c = [], []
    for c in range(NCORES):
        r = results[c]
        b8_inv = -np.asarray(r["b8"]).astype(np.float32).reshape(384, NB8)
        # c8 = 5-block sliding cover max of b8 (bf16 max is exact)
        b8p = np.full((384, NB8 + 4), -np.float32(1e30), np.float32)
        b8p[:, 2 : NB8 + 2] = b8_inv
        c8_inv = np.maximum.reduce(
            [b8p[:, i : i + NB8] for i in range(5)])
        zse = np.asarray(r["zse"]).astype(np.float32).reshape(384, NB8)
        bnd = np.log(zse) / C_LN + B0
        flag = b8_inv >= np.maximum(THR - 0.02, bnd - guard_f)
        t_, c_ = _resolve_core(flag, c8_inv, bnd, traces, nbr,
                               starts[c], c * INT, guard_h)
        all_t.append(t_)
        all_c.append(c_)

    times = np.concatenate(all_t) if all_t else np.empty(0, np.int64)
    chans = np.concatenate(all_c) if all_c else np.empty(0, np.int64)
    times, chans = times[:MAX_DET], chans[:MAX_DET]
    out_t = np.full(MAX_DET, -1, np.int64)
    out_c = np.full(MAX_DET, -1, np.int32)
    out_t[: times.size] = times
    out_c[: chans.size] = chans
    return out_t, out_c
